# revision 33
# baseline (speedup 1.0000x reference)
"""Trainium2 Bass kernel for nn_MemEffAttn (T=1024, B=4, E=1024, H=16, D=64).

Sharding (8 cores): head-parallel attention (2 heads x 4 batches per core),
Megatron-style column-sharded Wq/Wk/Wv, row-sharded Wo.  Each core computes a
full-shape partial of the output projection; the host sums the 8 partials
(row-parallel "gather") and reshapes to (T, B, E).

Per-core dataflow (all on-chip except noted):
  1. qT/kT projections emitted *transposed* ([dims, tokens], dims on
     partitions) so the head_dim contraction of the attention matmuls needs no
     on-device transposes at all; v is emitted in natural layout ([tokens,
     dims]) to serve as the stationary operand of P@V.
  2. RoPE applied in transposed layout with precomputed cos/sin tables
     (attention scale folded into q's tables).
  3. Scores are computed transposed (sT[k, tq] = kT.T @ qT), bias added from a
     host-transposed attn_bias shard, exp on ACT without max-subtraction
     (logits are O(10), fp32 exp is exact enough), then oT = v.T @ p
     accumulates over k-blocks.  A ones-column appended to v yields the
     softmax denominator for free; the key-padding mask is folded into v rows
     so masked keys drop out of both numerator and denominator.
  4. Output projection emitted transposed ([e, tokens]) so bo is a
     per-partition ACT bias; DMA'd out as a [1024, 4096] partial.
"""

import os
import sys

for _p in ("/opt/trn_rl_repo", "/root/.axon_site/_ro/trn_rl_repo"):
    if os.path.isdir(_p) and _p not in sys.path:
        sys.path.insert(0, _p)

import numpy as np
from contextlib import ExitStack

import concourse.bass as bass
import concourse.bacc as bacc
import concourse.tile as tile
from concourse import mybir
from concourse.bass_utils import run_bass_kernel_spmd

F32 = mybir.dt.float32
U8 = mybir.dt.uint8

E = 1024
H = 16
D = 64
T = 1024
B = 4
P = 128
NCORES = 8
HPC = H // NCORES  # heads per core = 2
TB = T * B  # 4096 tokens, stored b-major on device
NT = TB // 512  # 8 token tiles of 512
SCALE = 1.0 / np.sqrt(np.float32(D))  # 0.125

# matmul dtype knob: "f32" (safe) or "f32r" (full-rate, reduced precision)
MM_DTYPE = os.environ.get("KERNEL_MM_DTYPE", "f32r")
DEBUG_TAPS = os.environ.get("KERNEL_DEBUG", "") == "1"


MMDT = mybir.dt.float32r if MM_DTYPE == "f32r" else F32


def _mm(ap):
    return ap


def _build_bass():
    nc = bacc.Bacc("TRN2", target_bir_lowering=False, debug=False)

    # ---- per-core external inputs ----
    queryT = nc.dram_tensor("queryT", [E, TB], F32, kind="ExternalInput")
    biasT = nc.dram_tensor("biasT", [B * HPC, T, T], F32, kind="ExternalInput")
    wqT = nc.dram_tensor("wqT", [E, P], F32, kind="ExternalInput")
    wkT = nc.dram_tensor("wkT", [E, P], F32, kind="ExternalInput")
    wqsT = nc.dram_tensor("wqsT", [E, P], F32, kind="ExternalInput")
    wksT = nc.dram_tensor("wksT", [E, P], F32, kind="ExternalInput")
    wvT = nc.dram_tensor("wvT", [E, P], F32, kind="ExternalInput")
    woT = nc.dram_tensor("woT", [P, E], F32, kind="ExternalInput")
    bq_in = nc.dram_tensor("bq", [P, 1], F32, kind="ExternalInput")
    bqs_in = nc.dram_tensor("bqs", [P, 1], F32, kind="ExternalInput")
    bv_in = nc.dram_tensor("bv", [1, P], F32, kind="ExternalInput")
    bo_in = nc.dram_tensor("bo", [P, 8], F32, kind="ExternalInput")
    mask_in = nc.dram_tensor("masku8", [B, T], U8, kind="ExternalInput")
    cos_k = nc.dram_tensor("cos_k", [P, T], F32, kind="ExternalInput")
    sin_k = nc.dram_tensor("sin_k", [P, T], F32, kind="ExternalInput")
    outT = nc.dram_tensor("outT", [E, TB], F32, kind="ExternalOutput")
    dbg = {}
    if DEBUG_TAPS:
        for name, shape in (
            ("dbg_keep", [P, TB // P]),
            ("dbg_v", [P, 2 * (D + 2)]),
            ("dbg_qT", [P, 512]),
            ("dbg_kT", [P, 512]),
            ("dbg_l", [B * HPC, T]),
            ("dbg_rcp", [B * HPC, T]),
            ("dbg_p", [P, T]),
            ("dbg_s", [P, T]),
        ):
            dbg[name] = nc.dram_tensor(name, shape, F32, kind="ExternalOutput")

    Exp = mybir.ActivationFunctionType.Exp
    Identity = mybir.ActivationFunctionType.Identity
    Aadd = mybir.AluOpType.add
    Amul = mybir.AluOpType.mult

    with tile.TileContext(nc) as tc, ExitStack() as ctx:
        # ---------------- persistent tiles + constants ----------------
        persist = ctx.enter_context(tc.tile_pool(name="persist", bufs=1))
        # qT/kT/v/oT are rings over 2 batches (slot = b % 2): batch b's
        # attention reads slot b%2 while batch b+1's projections fill the
        # other slot.  Halves SBUF for the big activations.
        qT_sb = persist.tile([P, 2 * T], MMDT)  # roped, scaled q^T (2 heads)
        kT_sb = persist.tile([P, 2 * T], MMDT)  # roped k^T
        # v in natural layout with a ones column per head:
        # [tok128, tile, 66*2]: cols 0:64 head0 dims, 64 ones, 66:130 head1,
        # 130 ones (pads 65/131 zeroed)
        v_sb = persist.tile([P, 16, 2 * (D + 2)], MMDT)
        oT_sb = persist.tile([P, 2 * T], MMDT)  # attention out^T
        wo_sb = persist.tile([P, 8, P], MMDT)
        bo_sb = persist.tile([P, 8], F32)
        ident_f32 = persist.tile([P, P], F32)
        ident = persist.tile([P, P], MMDT)

        _wdma = nc.gpsimd if MM_DTYPE == "f32r" else nc.sync

        consts = ctx.enter_context(tc.tile_pool(name="consts", bufs=1))
        wq_sb = consts.tile([P, 8, P], MMDT)
        wk_sb = consts.tile([P, 8, P], MMDT)
        wv_sb = consts.tile([P, 8, P], MMDT)
        wqs_sb = consts.tile([P, 8, P], MMDT)
        wks_sb = consts.tile([P, 8, P], MMDT)
        bq_sb = consts.tile([P, 1], F32)
        bqs_sb = consts.tile([P, 1], F32)
        bv_sb = consts.tile([P, P], F32)  # bv broadcast along partitions
        ck_sb = consts.tile([P, T], F32)
        sk_sb = consts.tile([P, T], F32)
        cq_sb, sq_sb = ck_sb, sk_sb  # q scale folded into Wq on the host
        masku8_sb = consts.tile([P, TB // P], U8)
        keepT = consts.tile([P, TB // P], F32)

        # ---------------- fused pipeline ----------------
        # Projections are emitted as small chunks interleaved into the
        # attention loop's issue stream, so the PE's in-order queue always has
        # independent matmul work while DVE adds bias / ACT runs exp.  PV
        # matmuls lag their k-block by one iteration for the same reason.
        qry_pool = ctx.enter_context(tc.tile_pool(name="qry", bufs=2))
        ptmp_pool = ctx.enter_context(tc.tile_pool(name="ptmp", bufs=2))
        bias_pool = ctx.enter_context(tc.tile_pool(name="sbias", bufs=3))
        s_pool = ctx.enter_context(tc.tile_pool(name="ssb", bufs=2))
        p_pool = ctx.enter_context(tc.tile_pool(name="pp", bufs=3))
        rcp_pool = ctx.enter_context(tc.tile_pool(name="rcp", bufs=2))
        rbc_pool = ctx.enter_context(tc.tile_pool(name="rbc", bufs=1))
        outb_pool = ctx.enter_context(tc.tile_pool(name="outb", bufs=2))
        pj_psum = ctx.enter_context(tc.tile_pool(name="pj_psum", bufs=2, space="PSUM"))
        s_psum = ctx.enter_context(tc.tile_pool(name="s_psum", bufs=2, space="PSUM"))
        o_psum = ctx.enter_context(tc.tile_pool(name="o_psum", bufs=1, space="PSUM"))

        qry_tiles = {}

        def emit_qry_dma(nt):
            qry = qry_pool.tile([P, 8, 512], MMDT, tag="qry")
            _wdma.dma_start(
                out=qry[:],
                in_=bass.AP(
                    tensor=queryT,
                    offset=nt * 512,
                    ap=[[TB, P], [P * TB, 8], [1, 512]],
                ),
            )
            qry_tiles[nt] = qry

        def proj_chunks(nt):
            """Generator of small projection work chunks for token tile nt."""
            sl = slice((nt % 4) * 512, (nt % 4) * 512 + 512)
            tsl = slice((nt * 512) % T, (nt * 512) % T + 512)
            qry = qry_tiles[nt]
            state = {}

            def mm8(ps, w_sb):
                for k in range(8):
                    nc.tensor.matmul(
                        ps[:],
                        lhsT=w_sb[:, k, :],
                        rhs=qry[:, k, :],
                        start=(k == 0),
                        stop=(k == 7),
                    )

            for which, wm_sb, ws_sb, bm, bs, csb, ssb, dst in (
                ("q", wq_sb, wqs_sb, bq_sb, bqs_sb, cq_sb, sq_sb, qT_sb),
                ("k", wk_sb, wks_sb, None, None, ck_sb, sk_sb, kT_sb),
            ):

                def c_main(wm_sb=wm_sb, which=which):
                    ps_m = pj_psum.tile([P, 512], F32, tag="pj", name=f"pm{which}")
                    state["m"] = ps_m
                    mm8(ps_m, wm_sb)

                def c_swap(ws_sb=ws_sb, which=which):
                    ps_s = pj_psum.tile([P, 512], F32, tag="pj", name=f"psw{which}")
                    state["s"] = ps_s
                    mm8(ps_s, ws_sb)

                def c_rope(bm=bm, bs=bs, csb=csb, ssb=ssb, dst=dst):
                    ps_m, ps_s = state["m"], state["s"]
                    tmp = ptmp_pool.tile([P, 512], F32, tag="tmp", name="tmp")
                    nc.vector.scalar_tensor_tensor(
                        out=tmp[:],
                        in0=ps_s[:],
                        scalar=0.0 if bs is None else bs[:],
                        in1=ssb[:, tsl],
                        op0=Aadd,
                        op1=Amul,
                    )
                    nc.vector.scalar_tensor_tensor(
                        out=dst[:, sl],
                        in0=ps_m[:],
                        scalar=0.0 if bm is None else bm[:],
                        in1=csb[:, tsl],
                        op0=Aadd,
                        op1=Amul,
                    )
                    nc.vector.tensor_add(dst[:, sl], dst[:, sl], tmp[:])

                yield c_main
                yield c_swap
                yield c_rope

            def c_vt():
                # v projected transposed ([dims, tokens]: rhs N=512 keeps the
                # f32r matmul at full rate), staged to SBUF for PE transposes
                ps_vt = pj_psum.tile([P, 512], F32, tag="pj", name="psvt")
                for k in range(8):
                    nc.tensor.matmul(
                        ps_vt[:],
                        lhsT=wv_sb[:, k, :],
                        rhs=qry[:, k, :],
                        start=(k == 0),
                        stop=(k == 7),
                    )
                vt_sb = ptmp_pool.tile([P, 512], MMDT, tag="vt", name="vt")
                nc.vector.tensor_copy(vt_sb[:], ps_vt[:])
                state["vt"] = vt_sb

            yield c_vt

            for j in range(4):

                def c_vtr(j=j):
                    ti = (nt % 4) * 4 + j
                    vt_sb = state["vt"]
                    psv = pj_psum.tile([P, P], MMDT, tag="pj", name="psv")
                    nc.tensor.transpose(
                        psv[:], vt_sb[:, j * P : (j + 1) * P], ident[:]
                    )
                    pv = psv[:].bitcast(F32)
                    nc.vector.tensor_add(v_sb[:, ti, 0:D], pv[:, 0:D], bv_sb[:, 0:D])
                    nc.vector.tensor_add(
                        v_sb[:, ti, D + 2 : 2 * D + 2],
                        pv[:, D : 2 * D],
                        bv_sb[:, D : 2 * D],
                    )
                    nc.vector.memset(
                        v_sb[:, ti, D : 2 * (D + 2) : D + 2].bitcast(F32), 1.0
                    )
                    nc.vector.memset(
                        v_sb[:, ti, D + 1 : 2 * (D + 2) : D + 2].bitcast(F32), 0.0
                    )
                    # fold key-padding mask into v rows and the ones column
                    nc.vector.tensor_scalar_mul(
                        v_sb[:, ti, :], v_sb[:, ti, :], keepT[:, ti : ti + 1]
                    )
                    if DEBUG_TAPS and ti == 0:
                        nc.sync.dma_start(
                            out=dbg["dbg_v"][:], in_=v_sb[:, 0, :].bitcast(F32)
                        )

                yield c_vtr

        pending = []  # entries: (tag, fn); tag = ("proj", nt) or ("out", b)

        def pump(n):
            for _ in range(n):
                if pending:
                    pending.pop(0)[1]()

        def pump_proj_upto(nt_max):
            """Drain every projection chunk for nt <= nt_max (and anything
            queued before them)."""
            while any(t[0] == "proj" and t[1] <= nt_max for t, _ in pending):
                pending.pop(0)[1]()

        # startup DMA order: the serial SWDGE (cast) queue gets qry0 + the
        # q/qs weights first so the first projection matmuls start ASAP;
        # everything else follows.  Small consts go on the sync HWDGE queue.
        emit_qry_dma(0)
        for w_sb, w_dram in ((wq_sb, wqT), (wqs_sb, wqsT)):
            _wdma.dma_start(
                out=w_sb[:], in_=w_dram.ap().rearrange("(c p) m -> p c m", p=P)
            )
        nc.sync.dma_start(out=bq_sb[:], in_=bq_in[:])
        nc.sync.dma_start(out=bqs_sb[:], in_=bqs_in[:])
        for t_sb, t_dram in ((ck_sb, cos_k), (sk_sb, sin_k)):
            nc.sync.dma_start(out=t_sb[:], in_=t_dram[:])
        for w_sb, w_dram in ((wk_sb, wkT), (wks_sb, wksT), (wv_sb, wvT)):
            _wdma.dma_start(
                out=w_sb[:], in_=w_dram.ap().rearrange("(c p) m -> p c m", p=P)
            )
        emit_qry_dma(1)
        _wdma.dma_start(out=wo_sb[:], in_=woT.ap().rearrange("p (c m) -> p c m", m=P))
        nc.sync.dma_start(out=bo_sb[:], in_=bo_in[:])
        nc.sync.dma_start(
            out=bv_sb[:], in_=bass.AP(tensor=bv_in, offset=0, ap=[[0, P], [1, P]])
        )
        # key padding mask -> keep factor, transposed: keepT[p, ti] =
        # 1 - mask[b, tc*128 + p] with ti = b*8 + tc (b-major token tiles)
        nc.sync.dma_start(
            out=masku8_sb[:],
            in_=bass.AP(tensor=mask_in, offset=0, ap=[[1, P], [T, B], [P, 8]]),
        )
        nc.vector.tensor_scalar(
            out=keepT[:],
            in0=masku8_sb[:],
            scalar1=-1.0,
            scalar2=1.0,
            op0=Amul,
            op1=Aadd,
        )
        from concourse.masks import make_identity

        make_identity(nc, ident_f32[:])
        nc.vector.tensor_copy(ident[:], ident_f32[:])
        if DEBUG_TAPS:
            nc.sync.dma_start(out=dbg["dbg_keep"][:], in_=keepT[:])

        # prologue: project batch 0's tokens (nt 0, 1) densely
        pending.extend((("proj", 0), c) for c in proj_chunks(0))
        pending.extend((("proj", 1), c) for c in proj_chunks(1))
        pump(len(pending))
        if DEBUG_TAPS:
            nc.sync.dma_start(out=dbg["dbg_qT"][:], in_=qT_sb[:, 0:512].bitcast(F32))
            nc.sync.dma_start(out=dbg["dbg_kT"][:], in_=kT_sb[:, 0:512].bitcast(F32))

        for b in range(B):
            rb = b % 2
            bsl = slice(rb * T, (rb + 1) * T)
            pump_proj_upto(2 * b + 1)  # this batch's q/k/v must be complete
            if b + 1 < B:
                emit_qry_dma(2 * b + 2)
                emit_qry_dma(2 * b + 3)
                pending.extend(
                    (("proj", 2 * b + 2), c) for c in proj_chunks(2 * b + 2)
                )
                pending.extend(
                    (("proj", 2 * b + 3), c) for c in proj_chunks(2 * b + 3)
                )
            for h in range(HPC):
                bh = b * HPC + h
                hsl = slice(h * D, (h + 1) * D)
                o_ps = o_psum.tile([P, T], F32, tag="ops", name="ops")
                lagged = None
                for kbp in range(2):  # bias DMAs batched: 4 k-blocks, 2 MB
                    bias_t = bias_pool.tile([P, 4, T], F32, tag="bias", name="bias")
                    nc.sync.dma_start(
                        out=bias_t[:],
                        in_=bass.AP(
                            tensor=biasT,
                            offset=bh * T * T + kbp * 4 * P * T,
                            ap=[[T, P], [P * T, 4], [1, T]],
                        ),
                    )
                    for j in range(4):
                        kb = kbp * 4 + j
                        s_ps = s_psum.tile([P, T], F32, tag="sps", name="sps")
                        for half in range(2):
                            nc.tensor.matmul(
                                s_ps[:, half * 512 : (half + 1) * 512],
                                lhsT=kT_sb[hsl, rb * T + kb * P : rb * T + (kb + 1) * P],
                                rhs=qT_sb[
                                    hsl, rb * T + half * 512 : rb * T + (half + 1) * 512
                                ],
                                start=True,
                                stop=True,
                            )
                        pump(1)  # keep the PE queue fed while DVE/ACT drain
                        s_sb = s_pool.tile([P, T], F32, tag="ssb", name="ssb")
                        nc.vector.tensor_add(s_sb[:], s_ps[:], bias_t[:, j, :])
                        p_t = p_pool.tile([P, T], MMDT, tag="pt", name="pt")
                        nc.scalar.activation(p_t[:], s_sb[:], Exp)
                        if DEBUG_TAPS and bh == 0 and kb == 0:
                            nc.sync.dma_start(out=dbg["dbg_s"][:], in_=s_sb[:])
                            nc.sync.dma_start(
                                out=dbg["dbg_p"][:], in_=p_t[:].bitcast(F32)
                            )
                        if lagged is not None:
                            pk, pt_prev = lagged
                            for half in range(2):
                                nc.tensor.matmul(
                                    o_ps[0 : D + 1, half * 512 : (half + 1) * 512],
                                    lhsT=v_sb[
                                        :,
                                        rb * 8 + pk,
                                        h * (D + 2) : h * (D + 2) + D + 1,
                                    ],
                                    rhs=pt_prev[:, half * 512 : (half + 1) * 512],
                                    start=(pk == 0),
                                    stop=(pk == 7),
                                )
                        lagged = (kb, p_t)
                pk, pt_prev = lagged
                for half in range(2):
                    nc.tensor.matmul(
                        o_ps[0 : D + 1, half * 512 : (half + 1) * 512],
                        lhsT=v_sb[:, rb * 8 + pk, h * (D + 2) : h * (D + 2) + D + 1],
                        rhs=pt_prev[:, half * 512 : (half + 1) * 512],
                        start=(pk == 0),
                        stop=(pk == 7),
                    )
                # fast unnormalized evict releases the o psum slot; the
                # reciprocal chain + in-place normalize run off the critical
                # path.  (l goes to SBUF partition 0 first: rcp_approx_fast
                # miscomputes on a partition-shifted PSUM input.)
                l_sb = rcp_pool.tile([1, T], F32, tag="lsb", name="lsb")
                nc.vector.tensor_copy(l_sb[:], o_ps[D : D + 1, :])
                nc.scalar.copy(oT_sb[hsl, bsl], o_ps[0:D, :])
                rcp_row = rcp_pool.tile([1, T], F32, tag="lsb", name="rrow")
                nc.vector.reciprocal_approx_fast(rcp_row[:], l_sb[:])
                rcp_b = rbc_pool.tile([P, T], F32, tag="rbc", name="rbc")
                nc.gpsimd.partition_broadcast(rcp_b[:], rcp_row[:])
                if DEBUG_TAPS:
                    nc.sync.dma_start(out=dbg["dbg_l"][bh : bh + 1, :], in_=l_sb[:])
                    nc.sync.dma_start(
                        out=dbg["dbg_rcp"][bh : bh + 1, :], in_=rcp_row[:]
                    )
                nc.vector.tensor_mul(
                    oT_sb[hsl, bsl], oT_sb[hsl, bsl].bitcast(F32), rcp_b[hsl, :]
                )
            # output projection for batch b: queued as pump chunks so it
            # fills the next batch's PE gaps (inline for the last batch)
            def outproj_chunks(b=b):
                # et-quads sharing one [P, 4, 512] tile -> 1 MB output DMAs
                for half in range(2):
                    for eq in range(2):

                        def c_out(half=half, eq=eq, b=b):
                            ob = outb_pool.tile([P, 4, 512], F32, tag="ob", name="ob")
                            for ei in range(4):
                                et = eq * 4 + ei
                                psf = pj_psum.tile(
                                    [P, 512], F32, tag="pj", name="psf"
                                )
                                nc.tensor.matmul(
                                    psf[:],
                                    lhsT=wo_sb[:, et, :],
                                    rhs=oT_sb[
                                        :,
                                        (b % 2) * T + half * 512 : (b % 2) * T
                                        + (half + 1) * 512,
                                    ],
                                    start=True,
                                    stop=True,
                                )
                                if et % 2 == 0:
                                    nc.scalar.activation(
                                        ob[:, ei, :],
                                        psf[:],
                                        Identity,
                                        bias=bo_sb[:, et : et + 1],
                                        scale=1.0,
                                    )
                                else:
                                    nc.vector.tensor_scalar_add(
                                        ob[:, ei, :], psf[:], bo_sb[:, et : et + 1]
                                    )
                            nc.sync.dma_start(
                                out=bass.AP(
                                    tensor=outT,
                                    offset=eq * 4 * P * TB + b * T + half * 512,
                                    ap=[[TB, P], [P * TB, 4], [1, 512]],
                                ),
                                in_=ob[:],
                            )

                        yield c_out

            if b < B - 1:
                pending.extend((("out", b), c) for c in outproj_chunks())
            else:
                pump(len(pending))
                for c in outproj_chunks():
                    c()

    nc.compile()
    return nc


_NC_CACHE = None


def _get_nc():
    global _NC_CACHE
    if _NC_CACHE is None:
        _NC_CACHE = _build_bass()
    return _NC_CACHE


def _rope_tables():
    """cos/sin tables in [dim(128, 2 heads stacked), t] layout.

    Rows 0:32 of each 64-row head block carry -sin, rows 32:64 carry +sin
    (the rotate_half signs, indexed by output row: the swapped projection
    supplies qs[d] = q[partner(d)]).  q tables are pre-scaled by the
    attention scale.
    """
    d = np.arange(0, D, 2, dtype=np.float32) / np.float32(D)
    inv_freq = (np.float32(1.0) / np.power(np.float32(10000.0), d)).astype(np.float32)
    t = np.arange(T, dtype=np.float32)
    freqs = t[None, :] * inv_freq[:, None]  # [32, T]
    cos_h = np.cos(np.concatenate([freqs, freqs], axis=0)).astype(np.float32)  # [64,T]
    sin_half = np.sin(freqs).astype(np.float32)
    sin_signed = np.concatenate([-sin_half, sin_half], axis=0)  # [64, T]
    cos = np.vstack([cos_h, cos_h])  # [128, T] (2 heads)
    sin = np.vstack([sin_signed, sin_signed])
    return (np.ascontiguousarray(cos), np.ascontiguousarray(sin))


# partner-row permutation for the swapped projection: within each 64-dim
# head block, row d maps to (d+32) % 64
_SWAP = np.concatenate(
    [np.arange(64).reshape(2, 32)[::-1].ravel() + 64 * hh for hh in range(2)]
)


def _make_in_maps(query, attn_bias, key_padding_mask, Wq, bq, Wk, Wv, bv, Wo, bo):
    query = np.asarray(query, dtype=np.float32)
    attn_bias = np.asarray(attn_bias, dtype=np.float32)
    key_padding_mask = np.asarray(key_padding_mask)
    Wq = np.asarray(Wq, dtype=np.float32)
    Wk = np.asarray(Wk, dtype=np.float32)
    Wv = np.asarray(Wv, dtype=np.float32)
    Wo = np.asarray(Wo, dtype=np.float32)
    bq = np.asarray(bq, dtype=np.float32)
    bv = np.asarray(bv, dtype=np.float32)
    bo = np.asarray(bo, dtype=np.float32)

    # shared across cores
    queryT = np.ascontiguousarray(query.transpose(2, 1, 0).reshape(E, TB))
    masku8 = np.ascontiguousarray(key_padding_mask.astype(np.uint8))
    cos_k, sin_k = _rope_tables()
    bo_zero = np.zeros((P, 8), dtype=np.float32)
    bo_col = np.ascontiguousarray(bo.reshape(8, P).T)  # [p, echunk]

    in_maps = []
    for c in range(NCORES):
        rsl = slice(c * P, (c + 1) * P)
        in_maps.append(
            {
                "queryT": queryT,
                "biasT": np.ascontiguousarray(
                    attn_bias[:, c * HPC : (c + 1) * HPC].transpose(0, 1, 3, 2)
                ).reshape(B * HPC, T, T),
                "wqT": np.ascontiguousarray(Wq[rsl, :].T * np.float32(SCALE)),
                "wkT": np.ascontiguousarray(Wk[rsl, :].T),
                "wqsT": np.ascontiguousarray(Wq[rsl, :][_SWAP, :].T * np.float32(SCALE)),
                "wksT": np.ascontiguousarray(Wk[rsl, :][_SWAP, :].T),
                "wvT": np.ascontiguousarray(Wv[rsl, :].T),
                "woT": np.ascontiguousarray(Wo[:, rsl].T),
                "bq": np.ascontiguousarray(bq[rsl].reshape(P, 1) * np.float32(SCALE)),
                "bqs": np.ascontiguousarray(
                    bq[rsl][_SWAP].reshape(P, 1) * np.float32(SCALE)
                ),
                "bv": np.ascontiguousarray(bv[rsl].reshape(1, P)),
                "bo": bo_col if c == 0 else bo_zero,
                "masku8": masku8,
                "cos_k": cos_k,
                "sin_k": sin_k,
            }
        )
    return in_maps


def _run(inputs, trace=False, **kwargs):
    nc = _get_nc()
    in_maps = _make_in_maps(**inputs)
    res = run_bass_kernel_spmd(
        nc, in_maps, core_ids=list(range(NCORES)), trace=trace, **kwargs
    )
    acc = np.zeros((E, TB), dtype=np.float32)
    for r in res.results:
        acc += r["outT"]
    out = np.ascontiguousarray(acc.reshape(E, B, T).transpose(2, 1, 0))
    return out, res


def kernel(**inputs) -> np.ndarray:
    out, _ = _run(inputs, trace=False)
    return out


# revision 34
# speedup vs baseline: 1.0108x; 1.0108x over previous
"""Trainium2 Bass kernel for nn_MemEffAttn (T=1024, B=4, E=1024, H=16, D=64).

Sharding (8 cores): head-parallel attention (2 heads x 4 batches per core),
Megatron-style column-sharded Wq/Wk/Wv, row-sharded Wo.  Each core computes a
full-shape partial of the output projection; the host sums the 8 partials
(row-parallel "gather") and reshapes to (T, B, E).

Per-core dataflow (all on-chip except noted):
  1. qT/kT projections emitted *transposed* ([dims, tokens], dims on
     partitions) so the head_dim contraction of the attention matmuls needs no
     on-device transposes at all; v is emitted in natural layout ([tokens,
     dims]) to serve as the stationary operand of P@V.
  2. RoPE applied in transposed layout with precomputed cos/sin tables
     (attention scale folded into q's tables).
  3. Scores are computed transposed (sT[k, tq] = kT.T @ qT), bias added from a
     host-transposed attn_bias shard, exp on ACT without max-subtraction
     (logits are O(10), fp32 exp is exact enough), then oT = v.T @ p
     accumulates over k-blocks.  A ones-column appended to v yields the
     softmax denominator for free; the key-padding mask is folded into v rows
     so masked keys drop out of both numerator and denominator.
  4. Output projection emitted transposed ([e, tokens]) so bo is a
     per-partition ACT bias; DMA'd out as a [1024, 4096] partial.
"""

import os
import sys

for _p in ("/opt/trn_rl_repo", "/root/.axon_site/_ro/trn_rl_repo"):
    if os.path.isdir(_p) and _p not in sys.path:
        sys.path.insert(0, _p)

import numpy as np
from contextlib import ExitStack

import concourse.bass as bass
import concourse.bacc as bacc
import concourse.tile as tile
from concourse import mybir
from concourse.bass_utils import run_bass_kernel_spmd

F32 = mybir.dt.float32
U8 = mybir.dt.uint8

E = 1024
H = 16
D = 64
T = 1024
B = 4
P = 128
NCORES = 8
HPC = H // NCORES  # heads per core = 2
TB = T * B  # 4096 tokens, stored b-major on device
NT = TB // 512  # 8 token tiles of 512
SCALE = 1.0 / np.sqrt(np.float32(D))  # 0.125

# matmul dtype knob: "f32" (safe) or "f32r" (full-rate, reduced precision)
MM_DTYPE = os.environ.get("KERNEL_MM_DTYPE", "f32r")
DEBUG_TAPS = os.environ.get("KERNEL_DEBUG", "") == "1"


MMDT = mybir.dt.float32r if MM_DTYPE == "f32r" else F32


def _mm(ap):
    return ap


def _build_bass():
    nc = bacc.Bacc("TRN2", target_bir_lowering=False, debug=False)

    # ---- per-core external inputs ----
    queryT = nc.dram_tensor("queryT", [E, TB], F32, kind="ExternalInput")
    biasT = nc.dram_tensor("biasT", [B * HPC, T, T], F32, kind="ExternalInput")
    wqT = nc.dram_tensor("wqT", [E, P], F32, kind="ExternalInput")
    wkT = nc.dram_tensor("wkT", [E, P], F32, kind="ExternalInput")
    wqsT = nc.dram_tensor("wqsT", [E, P], F32, kind="ExternalInput")
    wksT = nc.dram_tensor("wksT", [E, P], F32, kind="ExternalInput")
    wvT = nc.dram_tensor("wvT", [E, P], F32, kind="ExternalInput")
    woT = nc.dram_tensor("woT", [P, E], F32, kind="ExternalInput")
    bq_in = nc.dram_tensor("bq", [P, 1], F32, kind="ExternalInput")
    bqs_in = nc.dram_tensor("bqs", [P, 1], F32, kind="ExternalInput")
    bv_in = nc.dram_tensor("bv", [1, P], F32, kind="ExternalInput")
    bo_in = nc.dram_tensor("bo", [P, 8], F32, kind="ExternalInput")
    mask_in = nc.dram_tensor("masku8", [B, T], U8, kind="ExternalInput")
    cos_k = nc.dram_tensor("cos_k", [P, T], F32, kind="ExternalInput")
    sin_k = nc.dram_tensor("sin_k", [P, T], F32, kind="ExternalInput")
    outT = nc.dram_tensor("outT", [E, TB], F32, kind="ExternalOutput")
    dbg = {}
    if DEBUG_TAPS:
        for name, shape in (
            ("dbg_keep", [P, TB // P]),
            ("dbg_v", [P, 2 * (D + 2)]),
            ("dbg_qT", [P, 512]),
            ("dbg_kT", [P, 512]),
            ("dbg_l", [B * HPC, T]),
            ("dbg_rcp", [B * HPC, T]),
            ("dbg_p", [P, T]),
            ("dbg_s", [P, T]),
        ):
            dbg[name] = nc.dram_tensor(name, shape, F32, kind="ExternalOutput")

    Exp = mybir.ActivationFunctionType.Exp
    Identity = mybir.ActivationFunctionType.Identity
    Aadd = mybir.AluOpType.add
    Amul = mybir.AluOpType.mult

    with tile.TileContext(nc) as tc, ExitStack() as ctx:
        # ---------------- persistent tiles + constants ----------------
        persist = ctx.enter_context(tc.tile_pool(name="persist", bufs=1))
        # qT/kT/v/oT are rings over 2 batches (slot = b % 2): batch b's
        # attention reads slot b%2 while batch b+1's projections fill the
        # other slot.  Halves SBUF for the big activations.
        qT_sb = persist.tile([P, 2 * T], MMDT)  # roped, scaled q^T (2 heads)
        kT_sb = persist.tile([P, 2 * T], MMDT)  # roped k^T
        # v in natural layout with a ones column per head:
        # [tok128, tile, 66*2]: cols 0:64 head0 dims, 64 ones, 66:130 head1,
        # 130 ones (pads 65/131 zeroed)
        v_sb = persist.tile([P, 16, 2 * (D + 2)], MMDT)
        oT_sb = persist.tile([P, 2 * T], MMDT)  # attention out^T
        wo_sb = persist.tile([P, 8, P], MMDT)
        bo_sb = persist.tile([P, 8], F32)
        ident_f32 = persist.tile([P, P], F32)
        ident = persist.tile([P, P], MMDT)

        _wdma = nc.gpsimd if MM_DTYPE == "f32r" else nc.sync

        consts = ctx.enter_context(tc.tile_pool(name="consts", bufs=1))
        wq_sb = consts.tile([P, 8, P], MMDT)
        wk_sb = consts.tile([P, 8, P], MMDT)
        wv_sb = consts.tile([P, 8, P], MMDT)
        wqs_sb = consts.tile([P, 8, P], MMDT)
        wks_sb = consts.tile([P, 8, P], MMDT)
        bq_sb = consts.tile([P, 1], F32)
        bqs_sb = consts.tile([P, 1], F32)
        bv_sb = consts.tile([P, P], F32)  # bv broadcast along partitions
        ck_sb = consts.tile([P, T], F32)
        sk_sb = consts.tile([P, T], F32)
        cq_sb, sq_sb = ck_sb, sk_sb  # q scale folded into Wq on the host
        masku8_sb = consts.tile([P, TB // P], U8)
        keepT = consts.tile([P, TB // P], F32)

        # ---------------- fused pipeline ----------------
        # Projections are emitted as small chunks interleaved into the
        # attention loop's issue stream, so the PE's in-order queue always has
        # independent matmul work while DVE adds bias / ACT runs exp.  PV
        # matmuls lag their k-block by one iteration for the same reason.
        qry_pool = ctx.enter_context(tc.tile_pool(name="qry", bufs=2))
        ptmp_pool = ctx.enter_context(tc.tile_pool(name="ptmp", bufs=2))
        bias_pool = ctx.enter_context(tc.tile_pool(name="sbias", bufs=3))
        s_pool = ctx.enter_context(tc.tile_pool(name="ssb", bufs=2))
        p_pool = ctx.enter_context(tc.tile_pool(name="pp", bufs=3))
        rcp_pool = ctx.enter_context(tc.tile_pool(name="rcp", bufs=2))
        rbc_pool = ctx.enter_context(tc.tile_pool(name="rbc", bufs=1))
        outb_pool = ctx.enter_context(tc.tile_pool(name="outb", bufs=2))
        pj_psum = ctx.enter_context(tc.tile_pool(name="pj_psum", bufs=2, space="PSUM"))
        s_psum = ctx.enter_context(tc.tile_pool(name="s_psum", bufs=2, space="PSUM"))
        o_psum = ctx.enter_context(tc.tile_pool(name="o_psum", bufs=1, space="PSUM"))

        qry_tiles = {}

        def emit_qry_dma(nt):
            qry = qry_pool.tile([P, 8, 512], MMDT, tag="qry")
            _wdma.dma_start(
                out=qry[:],
                in_=bass.AP(
                    tensor=queryT,
                    offset=nt * 512,
                    ap=[[TB, P], [P * TB, 8], [1, 512]],
                ),
            )
            qry_tiles[nt] = qry

        def proj_chunks(nt):
            """Generator of small projection work chunks for token tile nt."""
            sl = slice((nt % 4) * 512, (nt % 4) * 512 + 512)
            tsl = slice((nt * 512) % T, (nt * 512) % T + 512)
            qry = qry_tiles[nt]
            state = {}

            def mm8(ps, w_sb):
                for k in range(8):
                    nc.tensor.matmul(
                        ps[:],
                        lhsT=w_sb[:, k, :],
                        rhs=qry[:, k, :],
                        start=(k == 0),
                        stop=(k == 7),
                    )

            for which, wm_sb, ws_sb, bm, bs, csb, ssb, dst in (
                ("q", wq_sb, wqs_sb, bq_sb, bqs_sb, cq_sb, sq_sb, qT_sb),
                ("k", wk_sb, wks_sb, None, None, ck_sb, sk_sb, kT_sb),
            ):

                def c_main(wm_sb=wm_sb, which=which):
                    ps_m = pj_psum.tile([P, 512], F32, tag="pj", name=f"pm{which}")
                    state["m"] = ps_m
                    mm8(ps_m, wm_sb)

                def c_swap(ws_sb=ws_sb, which=which):
                    ps_s = pj_psum.tile([P, 512], F32, tag="pj", name=f"psw{which}")
                    state["s"] = ps_s
                    mm8(ps_s, ws_sb)

                def c_rope(bm=bm, bs=bs, csb=csb, ssb=ssb, dst=dst):
                    ps_m, ps_s = state["m"], state["s"]
                    tmp = ptmp_pool.tile([P, 512], F32, tag="tmp", name="tmp")
                    nc.vector.scalar_tensor_tensor(
                        out=tmp[:],
                        in0=ps_s[:],
                        scalar=0.0 if bs is None else bs[:],
                        in1=ssb[:, tsl],
                        op0=Aadd,
                        op1=Amul,
                    )
                    nc.vector.scalar_tensor_tensor(
                        out=dst[:, sl],
                        in0=ps_m[:],
                        scalar=0.0 if bm is None else bm[:],
                        in1=csb[:, tsl],
                        op0=Aadd,
                        op1=Amul,
                    )
                    nc.vector.tensor_add(dst[:, sl], dst[:, sl], tmp[:])

                yield c_main
                yield c_swap
                yield c_rope

            def c_vt():
                # v projected transposed ([dims, tokens]: rhs N=512 keeps the
                # f32r matmul at full rate), staged to SBUF for PE transposes
                ps_vt = pj_psum.tile([P, 512], F32, tag="pj", name="psvt")
                for k in range(8):
                    nc.tensor.matmul(
                        ps_vt[:],
                        lhsT=wv_sb[:, k, :],
                        rhs=qry[:, k, :],
                        start=(k == 0),
                        stop=(k == 7),
                    )
                vt_sb = ptmp_pool.tile([P, 512], MMDT, tag="vt", name="vt")
                nc.vector.tensor_copy(vt_sb[:], ps_vt[:])
                state["vt"] = vt_sb

            yield c_vt

            for j in range(4):

                def c_vtr(j=j):
                    ti = (nt % 4) * 4 + j
                    vt_sb = state["vt"]
                    psv = pj_psum.tile([P, P], MMDT, tag="pj", name="psv")
                    nc.tensor.transpose(
                        psv[:], vt_sb[:, j * P : (j + 1) * P], ident[:]
                    )
                    pv = psv[:].bitcast(F32)
                    nc.vector.tensor_add(v_sb[:, ti, 0:D], pv[:, 0:D], bv_sb[:, 0:D])
                    nc.vector.tensor_add(
                        v_sb[:, ti, D + 2 : 2 * D + 2],
                        pv[:, D : 2 * D],
                        bv_sb[:, D : 2 * D],
                    )
                    nc.vector.memset(
                        v_sb[:, ti, D : 2 * (D + 2) : D + 2].bitcast(F32), 1.0
                    )
                    nc.vector.memset(
                        v_sb[:, ti, D + 1 : 2 * (D + 2) : D + 2].bitcast(F32), 0.0
                    )
                    # fold key-padding mask into v rows and the ones column
                    nc.vector.tensor_scalar_mul(
                        v_sb[:, ti, :], v_sb[:, ti, :], keepT[:, ti : ti + 1]
                    )
                    if DEBUG_TAPS and ti == 0:
                        nc.sync.dma_start(
                            out=dbg["dbg_v"][:], in_=v_sb[:, 0, :].bitcast(F32)
                        )

                yield c_vtr

        pending = []  # entries: (tag, fn); tag = ("proj", nt) or ("out", b)

        def pump(n):
            for _ in range(n):
                if pending:
                    pending.pop(0)[1]()

        def pump_proj_upto(nt_max):
            """Drain every projection chunk for nt <= nt_max (and anything
            queued before them)."""
            while any(t[0] == "proj" and t[1] <= nt_max for t, _ in pending):
                pending.pop(0)[1]()

        # startup DMA order: the serial SWDGE (cast) queue gets qry0 + the
        # q/qs weights first so the first projection matmuls start ASAP;
        # everything else follows.  Small consts go on the sync HWDGE queue.
        emit_qry_dma(0)
        for w_sb, w_dram in ((wq_sb, wqT), (wqs_sb, wqsT)):
            _wdma.dma_start(
                out=w_sb[:], in_=w_dram.ap().rearrange("(c p) m -> p c m", p=P)
            )
        nc.sync.dma_start(out=bq_sb[:], in_=bq_in[:])
        nc.sync.dma_start(out=bqs_sb[:], in_=bqs_in[:])
        for t_sb, t_dram in ((ck_sb, cos_k), (sk_sb, sin_k)):
            nc.sync.dma_start(out=t_sb[:], in_=t_dram[:])
        for w_sb, w_dram in ((wk_sb, wkT), (wks_sb, wksT), (wv_sb, wvT)):
            _wdma.dma_start(
                out=w_sb[:], in_=w_dram.ap().rearrange("(c p) m -> p c m", p=P)
            )
        emit_qry_dma(1)
        _wdma.dma_start(out=wo_sb[:], in_=woT.ap().rearrange("p (c m) -> p c m", m=P))
        nc.sync.dma_start(out=bo_sb[:], in_=bo_in[:])
        nc.sync.dma_start(
            out=bv_sb[:], in_=bass.AP(tensor=bv_in, offset=0, ap=[[0, P], [1, P]])
        )
        # key padding mask -> keep factor, transposed: keepT[p, ti] =
        # 1 - mask[b, tc*128 + p] with ti = b*8 + tc (b-major token tiles)
        nc.sync.dma_start(
            out=masku8_sb[:],
            in_=bass.AP(tensor=mask_in, offset=0, ap=[[1, P], [T, B], [P, 8]]),
        )
        nc.vector.tensor_scalar(
            out=keepT[:],
            in0=masku8_sb[:],
            scalar1=-1.0,
            scalar2=1.0,
            op0=Amul,
            op1=Aadd,
        )
        from concourse.masks import make_identity

        make_identity(nc, ident_f32[:])
        nc.vector.tensor_copy(ident[:], ident_f32[:])
        if DEBUG_TAPS:
            nc.sync.dma_start(out=dbg["dbg_keep"][:], in_=keepT[:])

        # prologue: project batch 0's tokens (nt 0, 1) densely
        pending.extend((("proj", 0), c) for c in proj_chunks(0))
        pending.extend((("proj", 1), c) for c in proj_chunks(1))
        pump(len(pending))
        if DEBUG_TAPS:
            nc.sync.dma_start(out=dbg["dbg_qT"][:], in_=qT_sb[:, 0:512].bitcast(F32))
            nc.sync.dma_start(out=dbg["dbg_kT"][:], in_=kT_sb[:, 0:512].bitcast(F32))

        for b in range(B):
            rb = b % 2
            bsl = slice(rb * T, (rb + 1) * T)
            pump_proj_upto(2 * b + 1)  # this batch's q/k/v must be complete
            if b + 1 < B:
                emit_qry_dma(2 * b + 2)
                emit_qry_dma(2 * b + 3)
                pending.extend(
                    (("proj", 2 * b + 2), c) for c in proj_chunks(2 * b + 2)
                )
                pending.extend(
                    (("proj", 2 * b + 3), c) for c in proj_chunks(2 * b + 3)
                )
            for h in range(HPC):
                bh = b * HPC + h
                hsl = slice(h * D, (h + 1) * D)
                o_ps = o_psum.tile([P, T], F32, tag="ops", name="ops")
                lagged = None
                for kbp in range(4):  # bias DMAs batched: 2 k-blocks, 1 MB
                    bias_t = bias_pool.tile([P, 2, T], F32, tag="bias", name="bias")
                    nc.sync.dma_start(
                        out=bias_t[:],
                        in_=bass.AP(
                            tensor=biasT,
                            offset=bh * T * T + kbp * 2 * P * T,
                            ap=[[T, P], [P * T, 2], [1, T]],
                        ),
                    )
                    for j in range(2):
                        kb = kbp * 2 + j
                        s_ps = s_psum.tile([P, T], F32, tag="sps", name="sps")
                        for half in range(2):
                            nc.tensor.matmul(
                                s_ps[:, half * 512 : (half + 1) * 512],
                                lhsT=kT_sb[hsl, rb * T + kb * P : rb * T + (kb + 1) * P],
                                rhs=qT_sb[
                                    hsl, rb * T + half * 512 : rb * T + (half + 1) * 512
                                ],
                                start=True,
                                stop=True,
                            )
                        pump(1)  # keep the PE queue fed while DVE/ACT drain
                        s_sb = s_pool.tile([P, T], F32, tag="ssb", name="ssb")
                        nc.vector.tensor_add(s_sb[:], s_ps[:], bias_t[:, j, :])
                        p_t = p_pool.tile([P, T], MMDT, tag="pt", name="pt")
                        nc.scalar.activation(p_t[:], s_sb[:], Exp)
                        if DEBUG_TAPS and bh == 0 and kb == 0:
                            nc.sync.dma_start(out=dbg["dbg_s"][:], in_=s_sb[:])
                            nc.sync.dma_start(
                                out=dbg["dbg_p"][:], in_=p_t[:].bitcast(F32)
                            )
                        if lagged is not None:
                            pk, pt_prev = lagged
                            for half in range(2):
                                nc.tensor.matmul(
                                    o_ps[0 : D + 1, half * 512 : (half + 1) * 512],
                                    lhsT=v_sb[
                                        :,
                                        rb * 8 + pk,
                                        h * (D + 2) : h * (D + 2) + D + 1,
                                    ],
                                    rhs=pt_prev[:, half * 512 : (half + 1) * 512],
                                    start=(pk == 0),
                                    stop=(pk == 7),
                                )
                        lagged = (kb, p_t)
                pk, pt_prev = lagged
                for half in range(2):
                    nc.tensor.matmul(
                        o_ps[0 : D + 1, half * 512 : (half + 1) * 512],
                        lhsT=v_sb[:, rb * 8 + pk, h * (D + 2) : h * (D + 2) + D + 1],
                        rhs=pt_prev[:, half * 512 : (half + 1) * 512],
                        start=(pk == 0),
                        stop=(pk == 7),
                    )
                # fast unnormalized evict releases the o psum slot; the
                # reciprocal chain + in-place normalize run off the critical
                # path.  (l goes to SBUF partition 0 first: rcp_approx_fast
                # miscomputes on a partition-shifted PSUM input.)
                l_sb = rcp_pool.tile([1, T], F32, tag="lsb", name="lsb")
                nc.vector.tensor_copy(l_sb[:], o_ps[D : D + 1, :])
                nc.scalar.copy(oT_sb[hsl, bsl], o_ps[0:D, :])
                rcp_row = rcp_pool.tile([1, T], F32, tag="lsb", name="rrow")
                nc.vector.reciprocal_approx_fast(rcp_row[:], l_sb[:])
                rcp_b = rbc_pool.tile([P, T], F32, tag="rbc", name="rbc")
                nc.gpsimd.partition_broadcast(rcp_b[:], rcp_row[:])
                if DEBUG_TAPS:
                    nc.sync.dma_start(out=dbg["dbg_l"][bh : bh + 1, :], in_=l_sb[:])
                    nc.sync.dma_start(
                        out=dbg["dbg_rcp"][bh : bh + 1, :], in_=rcp_row[:]
                    )
                nc.vector.tensor_mul(
                    oT_sb[hsl, bsl], oT_sb[hsl, bsl].bitcast(F32), rcp_b[hsl, :]
                )
            # output projection for batch b: queued as pump chunks so it
            # fills the next batch's PE gaps (inline for the last batch)
            def outproj_chunks(b=b):
                # et-quads sharing one [P, 4, 512] tile -> 1 MB output DMAs
                for half in range(2):
                    for eq in range(2):

                        def c_out(half=half, eq=eq, b=b):
                            ob = outb_pool.tile([P, 4, 512], F32, tag="ob", name="ob")
                            for ei in range(4):
                                et = eq * 4 + ei
                                psf = pj_psum.tile(
                                    [P, 512], F32, tag="pj", name="psf"
                                )
                                nc.tensor.matmul(
                                    psf[:],
                                    lhsT=wo_sb[:, et, :],
                                    rhs=oT_sb[
                                        :,
                                        (b % 2) * T + half * 512 : (b % 2) * T
                                        + (half + 1) * 512,
                                    ],
                                    start=True,
                                    stop=True,
                                )
                                if et % 2 == 0:
                                    nc.scalar.activation(
                                        ob[:, ei, :],
                                        psf[:],
                                        Identity,
                                        bias=bo_sb[:, et : et + 1],
                                        scale=1.0,
                                    )
                                else:
                                    nc.vector.tensor_scalar_add(
                                        ob[:, ei, :], psf[:], bo_sb[:, et : et + 1]
                                    )
                            nc.sync.dma_start(
                                out=bass.AP(
                                    tensor=outT,
                                    offset=eq * 4 * P * TB + b * T + half * 512,
                                    ap=[[TB, P], [P * TB, 4], [1, 512]],
                                ),
                                in_=ob[:],
                            )

                        yield c_out

            if b < B - 1:
                pending.extend((("out", b), c) for c in outproj_chunks())
            else:
                pump(len(pending))
                for c in outproj_chunks():
                    c()

    nc.compile()
    return nc


_NC_CACHE = None


def _get_nc():
    global _NC_CACHE
    if _NC_CACHE is None:
        _NC_CACHE = _build_bass()
    return _NC_CACHE


def _rope_tables():
    """cos/sin tables in [dim(128, 2 heads stacked), t] layout.

    Rows 0:32 of each 64-row head block carry -sin, rows 32:64 carry +sin
    (the rotate_half signs, indexed by output row: the swapped projection
    supplies qs[d] = q[partner(d)]).  q tables are pre-scaled by the
    attention scale.
    """
    d = np.arange(0, D, 2, dtype=np.float32) / np.float32(D)
    inv_freq = (np.float32(1.0) / np.power(np.float32(10000.0), d)).astype(np.float32)
    t = np.arange(T, dtype=np.float32)
    freqs = t[None, :] * inv_freq[:, None]  # [32, T]
    cos_h = np.cos(np.concatenate([freqs, freqs], axis=0)).astype(np.float32)  # [64,T]
    sin_half = np.sin(freqs).astype(np.float32)
    sin_signed = np.concatenate([-sin_half, sin_half], axis=0)  # [64, T]
    cos = np.vstack([cos_h, cos_h])  # [128, T] (2 heads)
    sin = np.vstack([sin_signed, sin_signed])
    return (np.ascontiguousarray(cos), np.ascontiguousarray(sin))


# partner-row permutation for the swapped projection: within each 64-dim
# head block, row d maps to (d+32) % 64
_SWAP = np.concatenate(
    [np.arange(64).reshape(2, 32)[::-1].ravel() + 64 * hh for hh in range(2)]
)


def _make_in_maps(query, attn_bias, key_padding_mask, Wq, bq, Wk, Wv, bv, Wo, bo):
    query = np.asarray(query, dtype=np.float32)
    attn_bias = np.asarray(attn_bias, dtype=np.float32)
    key_padding_mask = np.asarray(key_padding_mask)
    Wq = np.asarray(Wq, dtype=np.float32)
    Wk = np.asarray(Wk, dtype=np.float32)
    Wv = np.asarray(Wv, dtype=np.float32)
    Wo = np.asarray(Wo, dtype=np.float32)
    bq = np.asarray(bq, dtype=np.float32)
    bv = np.asarray(bv, dtype=np.float32)
    bo = np.asarray(bo, dtype=np.float32)

    # shared across cores
    queryT = np.ascontiguousarray(query.transpose(2, 1, 0).reshape(E, TB))
    masku8 = np.ascontiguousarray(key_padding_mask.astype(np.uint8))
    cos_k, sin_k = _rope_tables()
    bo_zero = np.zeros((P, 8), dtype=np.float32)
    bo_col = np.ascontiguousarray(bo.reshape(8, P).T)  # [p, echunk]

    in_maps = []
    for c in range(NCORES):
        rsl = slice(c * P, (c + 1) * P)
        in_maps.append(
            {
                "queryT": queryT,
                "biasT": np.ascontiguousarray(
                    attn_bias[:, c * HPC : (c + 1) * HPC].transpose(0, 1, 3, 2)
                ).reshape(B * HPC, T, T),
                "wqT": np.ascontiguousarray(Wq[rsl, :].T * np.float32(SCALE)),
                "wkT": np.ascontiguousarray(Wk[rsl, :].T),
                "wqsT": np.ascontiguousarray(Wq[rsl, :][_SWAP, :].T * np.float32(SCALE)),
                "wksT": np.ascontiguousarray(Wk[rsl, :][_SWAP, :].T),
                "wvT": np.ascontiguousarray(Wv[rsl, :].T),
                "woT": np.ascontiguousarray(Wo[:, rsl].T),
                "bq": np.ascontiguousarray(bq[rsl].reshape(P, 1) * np.float32(SCALE)),
                "bqs": np.ascontiguousarray(
                    bq[rsl][_SWAP].reshape(P, 1) * np.float32(SCALE)
                ),
                "bv": np.ascontiguousarray(bv[rsl].reshape(1, P)),
                "bo": bo_col if c == 0 else bo_zero,
                "masku8": masku8,
                "cos_k": cos_k,
                "sin_k": sin_k,
            }
        )
    return in_maps


def _run(inputs, trace=False, **kwargs):
    nc = _get_nc()
    in_maps = _make_in_maps(**inputs)
    res = run_bass_kernel_spmd(
        nc, in_maps, core_ids=list(range(NCORES)), trace=trace, **kwargs
    )
    acc = np.zeros((E, TB), dtype=np.float32)
    for r in res.results:
        acc += r["outT"]
    out = np.ascontiguousarray(acc.reshape(E, B, T).transpose(2, 1, 0))
    return out, res


def kernel(**inputs) -> np.ndarray:
    out, _ = _run(inputs, trace=False)
    return out


# revision 35
# speedup vs baseline: 1.0302x; 1.0192x over previous
"""Trainium2 Bass kernel for nn_MemEffAttn (T=1024, B=4, E=1024, H=16, D=64).

Sharding (8 cores): head-parallel attention (2 heads x 4 batches per core),
Megatron-style column-sharded Wq/Wk/Wv, row-sharded Wo.  Each core computes a
full-shape partial of the output projection; the host sums the 8 partials
(row-parallel "gather") and reshapes to (T, B, E).

Per-core dataflow (all on-chip except noted):
  1. qT/kT projections emitted *transposed* ([dims, tokens], dims on
     partitions) so the head_dim contraction of the attention matmuls needs no
     on-device transposes at all; v is emitted in natural layout ([tokens,
     dims]) to serve as the stationary operand of P@V.
  2. RoPE applied in transposed layout with precomputed cos/sin tables
     (attention scale folded into q's tables).
  3. Scores are computed transposed (sT[k, tq] = kT.T @ qT), bias added from a
     host-transposed attn_bias shard, exp on ACT without max-subtraction
     (logits are O(10), fp32 exp is exact enough), then oT = v.T @ p
     accumulates over k-blocks.  A ones-column appended to v yields the
     softmax denominator for free; the key-padding mask is folded into v rows
     so masked keys drop out of both numerator and denominator.
  4. Output projection emitted transposed ([e, tokens]) so bo is a
     per-partition ACT bias; DMA'd out as a [1024, 4096] partial.
"""

import os
import sys

for _p in ("/opt/trn_rl_repo", "/root/.axon_site/_ro/trn_rl_repo"):
    if os.path.isdir(_p) and _p not in sys.path:
        sys.path.insert(0, _p)

import numpy as np
from contextlib import ExitStack

import concourse.bass as bass
import concourse.bacc as bacc
import concourse.tile as tile
from concourse import mybir
from concourse.bass_utils import run_bass_kernel_spmd

F32 = mybir.dt.float32
U8 = mybir.dt.uint8

E = 1024
H = 16
D = 64
T = 1024
B = 4
P = 128
NCORES = 8
HPC = H // NCORES  # heads per core = 2
TB = T * B  # 4096 tokens, stored b-major on device
NT = TB // 512  # 8 token tiles of 512
SCALE = 1.0 / np.sqrt(np.float32(D))  # 0.125

# matmul dtype knob: "f32" (safe) or "f32r" (full-rate, reduced precision)
MM_DTYPE = os.environ.get("KERNEL_MM_DTYPE", "f32r")
DEBUG_TAPS = os.environ.get("KERNEL_DEBUG", "") == "1"


MMDT = mybir.dt.float32r if MM_DTYPE == "f32r" else F32


def _mm(ap):
    return ap


def _build_bass():
    nc = bacc.Bacc("TRN2", target_bir_lowering=False, debug=False)

    # ---- per-core external inputs ----
    queryT = nc.dram_tensor("queryT", [E, TB], F32, kind="ExternalInput")
    biasT = nc.dram_tensor("biasT", [B * HPC, T, T], F32, kind="ExternalInput")
    wqT = nc.dram_tensor("wqT", [E, P], F32, kind="ExternalInput")
    wkT = nc.dram_tensor("wkT", [E, P], F32, kind="ExternalInput")
    wqsT = nc.dram_tensor("wqsT", [E, P], F32, kind="ExternalInput")
    wksT = nc.dram_tensor("wksT", [E, P], F32, kind="ExternalInput")
    wvT = nc.dram_tensor("wvT", [E, P], F32, kind="ExternalInput")
    woT = nc.dram_tensor("woT", [P, E], F32, kind="ExternalInput")
    bq_in = nc.dram_tensor("bq", [P, 1], F32, kind="ExternalInput")
    bqs_in = nc.dram_tensor("bqs", [P, 1], F32, kind="ExternalInput")
    bv_in = nc.dram_tensor("bv", [1, P], F32, kind="ExternalInput")
    bo_in = nc.dram_tensor("bo", [P, 8], F32, kind="ExternalInput")
    mask_in = nc.dram_tensor("masku8", [B, T], U8, kind="ExternalInput")
    cos_k = nc.dram_tensor("cos_k", [P, T], F32, kind="ExternalInput")
    sin_k = nc.dram_tensor("sin_k", [P, T], F32, kind="ExternalInput")
    outT = nc.dram_tensor("outT", [E, TB], F32, kind="ExternalOutput")
    dbg = {}
    if DEBUG_TAPS:
        for name, shape in (
            ("dbg_keep", [P, TB // P]),
            ("dbg_v", [P, 2 * (D + 2)]),
            ("dbg_qT", [P, 512]),
            ("dbg_kT", [P, 512]),
            ("dbg_l", [B * HPC, T]),
            ("dbg_rcp", [B * HPC, T]),
            ("dbg_p", [P, T]),
            ("dbg_s", [P, T]),
        ):
            dbg[name] = nc.dram_tensor(name, shape, F32, kind="ExternalOutput")

    Exp = mybir.ActivationFunctionType.Exp
    Identity = mybir.ActivationFunctionType.Identity
    Aadd = mybir.AluOpType.add
    Amul = mybir.AluOpType.mult

    with tile.TileContext(nc) as tc, ExitStack() as ctx:
        # ---------------- persistent tiles + constants ----------------
        persist = ctx.enter_context(tc.tile_pool(name="persist", bufs=1))
        # qT/kT/v/oT are rings over 2 batches (slot = b % 2): batch b's
        # attention reads slot b%2 while batch b+1's projections fill the
        # other slot.  Halves SBUF for the big activations.
        qT_sb = persist.tile([P, 2 * T], MMDT)  # roped, scaled q^T (2 heads)
        kT_sb = persist.tile([P, 2 * T], MMDT)  # roped k^T
        # v in natural layout with a ones column per head:
        # [tok128, tile, 66*2]: cols 0:64 head0 dims, 64 ones, 66:130 head1,
        # 130 ones (pads 65/131 zeroed)
        v_sb = persist.tile([P, 16, 2 * (D + 2)], MMDT)
        oT_sb = persist.tile([P, 2 * T], MMDT)  # attention out^T
        wo_sb = persist.tile([P, 8, P], MMDT)
        bo_sb = persist.tile([P, 8], F32)
        ident_f32 = persist.tile([P, P], F32)
        ident = persist.tile([P, P], MMDT)

        _wdma = nc.gpsimd if MM_DTYPE == "f32r" else nc.sync

        consts = ctx.enter_context(tc.tile_pool(name="consts", bufs=1))
        wq_sb = consts.tile([P, 8, P], MMDT)
        wk_sb = consts.tile([P, 8, P], MMDT)
        wv_sb = consts.tile([P, 8, P], MMDT)
        wqs_sb = consts.tile([P, 8, P], MMDT)
        wks_sb = consts.tile([P, 8, P], MMDT)
        bq_sb = consts.tile([P, 1], F32)
        bqs_sb = consts.tile([P, 1], F32)
        bv_sb = consts.tile([P, P], F32)  # bv broadcast along partitions
        ck_sb = consts.tile([P, T], F32)
        sk_sb = consts.tile([P, T], F32)
        cq_sb, sq_sb = ck_sb, sk_sb  # q scale folded into Wq on the host
        masku8_sb = consts.tile([P, TB // P], U8)
        keepT = consts.tile([P, TB // P], F32)

        # ---------------- fused pipeline ----------------
        # Projections are emitted as small chunks interleaved into the
        # attention loop's issue stream, so the PE's in-order queue always has
        # independent matmul work while DVE adds bias / ACT runs exp.  PV
        # matmuls lag their k-block by one iteration for the same reason.
        qry_pool = ctx.enter_context(tc.tile_pool(name="qry", bufs=2))
        ptmp_pool = ctx.enter_context(tc.tile_pool(name="ptmp", bufs=2))
        bias_pool = ctx.enter_context(tc.tile_pool(name="sbias", bufs=3))
        s_pool = ctx.enter_context(tc.tile_pool(name="ssb", bufs=2))
        p_pool = ctx.enter_context(tc.tile_pool(name="pp", bufs=3))
        rcp_pool = ctx.enter_context(tc.tile_pool(name="rcp", bufs=2))
        rbc_pool = ctx.enter_context(tc.tile_pool(name="rbc", bufs=1))
        outb_pool = ctx.enter_context(tc.tile_pool(name="outb", bufs=2))
        pj_psum = ctx.enter_context(tc.tile_pool(name="pj_psum", bufs=2, space="PSUM"))
        s_psum = ctx.enter_context(tc.tile_pool(name="s_psum", bufs=2, space="PSUM"))
        o_psum = ctx.enter_context(tc.tile_pool(name="o_psum", bufs=1, space="PSUM"))

        qry_tiles = {}

        def emit_qry_dma(nt):
            qry = qry_pool.tile([P, 8, 512], MMDT, tag="qry")
            for kh in range(2):
                _wdma.dma_start(
                    out=qry[:, kh * 4 : (kh + 1) * 4, :],
                    in_=bass.AP(
                        tensor=queryT,
                        offset=kh * 4 * P * TB + nt * 512,
                        ap=[[TB, P], [P * TB, 4], [1, 512]],
                    ),
                )
            qry_tiles[nt] = qry

        def proj_chunks(nt):
            """Generator of small projection work chunks for token tile nt."""
            sl = slice((nt % 4) * 512, (nt % 4) * 512 + 512)
            tsl = slice((nt * 512) % T, (nt * 512) % T + 512)
            qry = qry_tiles[nt]
            state = {}

            def mm8(ps, w_sb):
                for k in range(8):
                    nc.tensor.matmul(
                        ps[:],
                        lhsT=w_sb[:, k, :],
                        rhs=qry[:, k, :],
                        start=(k == 0),
                        stop=(k == 7),
                    )

            for which, wm_sb, ws_sb, bm, bs, csb, ssb, dst in (
                ("q", wq_sb, wqs_sb, bq_sb, bqs_sb, cq_sb, sq_sb, qT_sb),
                ("k", wk_sb, wks_sb, None, None, ck_sb, sk_sb, kT_sb),
            ):

                def c_main(wm_sb=wm_sb, which=which):
                    ps_m = pj_psum.tile([P, 512], F32, tag="pj", name=f"pm{which}")
                    state["m"] = ps_m
                    mm8(ps_m, wm_sb)

                def c_swap(ws_sb=ws_sb, which=which):
                    ps_s = pj_psum.tile([P, 512], F32, tag="pj", name=f"psw{which}")
                    state["s"] = ps_s
                    mm8(ps_s, ws_sb)

                def c_rope(bm=bm, bs=bs, csb=csb, ssb=ssb, dst=dst):
                    ps_m, ps_s = state["m"], state["s"]
                    tmp = ptmp_pool.tile([P, 512], F32, tag="tmp", name="tmp")
                    nc.vector.scalar_tensor_tensor(
                        out=tmp[:],
                        in0=ps_s[:],
                        scalar=0.0 if bs is None else bs[:],
                        in1=ssb[:, tsl],
                        op0=Aadd,
                        op1=Amul,
                    )
                    nc.vector.scalar_tensor_tensor(
                        out=dst[:, sl],
                        in0=ps_m[:],
                        scalar=0.0 if bm is None else bm[:],
                        in1=csb[:, tsl],
                        op0=Aadd,
                        op1=Amul,
                    )
                    nc.vector.tensor_add(dst[:, sl], dst[:, sl], tmp[:])

                yield c_main
                yield c_swap
                yield c_rope

            def c_vt():
                # v projected transposed ([dims, tokens]: rhs N=512 keeps the
                # f32r matmul at full rate), staged to SBUF for PE transposes
                ps_vt = pj_psum.tile([P, 512], F32, tag="pj", name="psvt")
                for k in range(8):
                    nc.tensor.matmul(
                        ps_vt[:],
                        lhsT=wv_sb[:, k, :],
                        rhs=qry[:, k, :],
                        start=(k == 0),
                        stop=(k == 7),
                    )
                vt_sb = ptmp_pool.tile([P, 512], MMDT, tag="vt", name="vt")
                nc.vector.tensor_copy(vt_sb[:], ps_vt[:])
                state["vt"] = vt_sb

            yield c_vt

            for j in range(4):

                def c_vtr(j=j):
                    ti = (nt % 4) * 4 + j
                    vt_sb = state["vt"]
                    psv = pj_psum.tile([P, P], MMDT, tag="pj", name="psv")
                    nc.tensor.transpose(
                        psv[:], vt_sb[:, j * P : (j + 1) * P], ident[:]
                    )
                    pv = psv[:].bitcast(F32)
                    nc.vector.tensor_add(v_sb[:, ti, 0:D], pv[:, 0:D], bv_sb[:, 0:D])
                    nc.vector.tensor_add(
                        v_sb[:, ti, D + 2 : 2 * D + 2],
                        pv[:, D : 2 * D],
                        bv_sb[:, D : 2 * D],
                    )
                    nc.vector.memset(
                        v_sb[:, ti, D : 2 * (D + 2) : D + 2].bitcast(F32), 1.0
                    )
                    nc.vector.memset(
                        v_sb[:, ti, D + 1 : 2 * (D + 2) : D + 2].bitcast(F32), 0.0
                    )
                    # fold key-padding mask into v rows and the ones column
                    nc.vector.tensor_scalar_mul(
                        v_sb[:, ti, :], v_sb[:, ti, :], keepT[:, ti : ti + 1]
                    )
                    if DEBUG_TAPS and ti == 0:
                        nc.sync.dma_start(
                            out=dbg["dbg_v"][:], in_=v_sb[:, 0, :].bitcast(F32)
                        )

                yield c_vtr

        pending = []  # entries: (tag, fn); tag = ("proj", nt) or ("out", b)

        def pump(n):
            for _ in range(n):
                if pending:
                    pending.pop(0)[1]()

        def pump_proj_upto(nt_max):
            """Drain every projection chunk for nt <= nt_max (and anything
            queued before them)."""
            while any(t[0] == "proj" and t[1] <= nt_max for t, _ in pending):
                pending.pop(0)[1]()

        # startup DMA order: the serial SWDGE (cast) queue gets qry0 + the
        # q/qs weights first so the first projection matmuls start ASAP;
        # everything else follows.  Small consts go on the sync HWDGE queue.
        emit_qry_dma(0)
        for w_sb, w_dram in ((wq_sb, wqT), (wqs_sb, wqsT)):
            _wdma.dma_start(
                out=w_sb[:], in_=w_dram.ap().rearrange("(c p) m -> p c m", p=P)
            )
        nc.sync.dma_start(out=bq_sb[:], in_=bq_in[:])
        nc.sync.dma_start(out=bqs_sb[:], in_=bqs_in[:])
        for t_sb, t_dram in ((ck_sb, cos_k), (sk_sb, sin_k)):
            nc.sync.dma_start(out=t_sb[:], in_=t_dram[:])
        for w_sb, w_dram in ((wk_sb, wkT), (wks_sb, wksT), (wv_sb, wvT)):
            _wdma.dma_start(
                out=w_sb[:], in_=w_dram.ap().rearrange("(c p) m -> p c m", p=P)
            )
        emit_qry_dma(1)
        _wdma.dma_start(out=wo_sb[:], in_=woT.ap().rearrange("p (c m) -> p c m", m=P))
        nc.sync.dma_start(out=bo_sb[:], in_=bo_in[:])
        nc.sync.dma_start(
            out=bv_sb[:], in_=bass.AP(tensor=bv_in, offset=0, ap=[[0, P], [1, P]])
        )
        # key padding mask -> keep factor, transposed: keepT[p, ti] =
        # 1 - mask[b, tc*128 + p] with ti = b*8 + tc (b-major token tiles)
        nc.sync.dma_start(
            out=masku8_sb[:],
            in_=bass.AP(tensor=mask_in, offset=0, ap=[[1, P], [T, B], [P, 8]]),
        )
        nc.vector.tensor_scalar(
            out=keepT[:],
            in0=masku8_sb[:],
            scalar1=-1.0,
            scalar2=1.0,
            op0=Amul,
            op1=Aadd,
        )
        from concourse.masks import make_identity

        make_identity(nc, ident_f32[:])
        nc.vector.tensor_copy(ident[:], ident_f32[:])
        if DEBUG_TAPS:
            nc.sync.dma_start(out=dbg["dbg_keep"][:], in_=keepT[:])

        # prologue: project batch 0's tokens (nt 0, 1) densely
        pending.extend((("proj", 0), c) for c in proj_chunks(0))
        pending.extend((("proj", 1), c) for c in proj_chunks(1))
        pump(len(pending))
        if DEBUG_TAPS:
            nc.sync.dma_start(out=dbg["dbg_qT"][:], in_=qT_sb[:, 0:512].bitcast(F32))
            nc.sync.dma_start(out=dbg["dbg_kT"][:], in_=kT_sb[:, 0:512].bitcast(F32))

        for b in range(B):
            rb = b % 2
            bsl = slice(rb * T, (rb + 1) * T)
            pump_proj_upto(2 * b + 1)  # this batch's q/k/v must be complete
            if b + 1 < B:
                emit_qry_dma(2 * b + 2)
                emit_qry_dma(2 * b + 3)
                pending.extend(
                    (("proj", 2 * b + 2), c) for c in proj_chunks(2 * b + 2)
                )
                pending.extend(
                    (("proj", 2 * b + 3), c) for c in proj_chunks(2 * b + 3)
                )
            for h in range(HPC):
                bh = b * HPC + h
                hsl = slice(h * D, (h + 1) * D)
                o_ps = o_psum.tile([P, T], F32, tag="ops", name="ops")
                lagged = None
                for kbp in range(4):  # bias DMAs batched: 2 k-blocks, 1 MB
                    bias_t = bias_pool.tile([P, 2, T], F32, tag="bias", name="bias")
                    nc.sync.dma_start(
                        out=bias_t[:],
                        in_=bass.AP(
                            tensor=biasT,
                            offset=bh * T * T + kbp * 2 * P * T,
                            ap=[[T, P], [P * T, 2], [1, T]],
                        ),
                    )
                    for j in range(2):
                        kb = kbp * 2 + j
                        s_ps = s_psum.tile([P, T], F32, tag="sps", name="sps")
                        for half in range(2):
                            nc.tensor.matmul(
                                s_ps[:, half * 512 : (half + 1) * 512],
                                lhsT=kT_sb[hsl, rb * T + kb * P : rb * T + (kb + 1) * P],
                                rhs=qT_sb[
                                    hsl, rb * T + half * 512 : rb * T + (half + 1) * 512
                                ],
                                start=True,
                                stop=True,
                            )
                        pump(1)  # keep the PE queue fed while DVE/ACT drain
                        s_sb = s_pool.tile([P, T], F32, tag="ssb", name="ssb")
                        nc.vector.tensor_add(s_sb[:], s_ps[:], bias_t[:, j, :])
                        p_t = p_pool.tile([P, T], MMDT, tag="pt", name="pt")
                        nc.scalar.activation(p_t[:], s_sb[:], Exp)
                        if DEBUG_TAPS and bh == 0 and kb == 0:
                            nc.sync.dma_start(out=dbg["dbg_s"][:], in_=s_sb[:])
                            nc.sync.dma_start(
                                out=dbg["dbg_p"][:], in_=p_t[:].bitcast(F32)
                            )
                        if lagged is not None:
                            pk, pt_prev = lagged
                            for half in range(2):
                                nc.tensor.matmul(
                                    o_ps[0 : D + 1, half * 512 : (half + 1) * 512],
                                    lhsT=v_sb[
                                        :,
                                        rb * 8 + pk,
                                        h * (D + 2) : h * (D + 2) + D + 1,
                                    ],
                                    rhs=pt_prev[:, half * 512 : (half + 1) * 512],
                                    start=(pk == 0),
                                    stop=(pk == 7),
                                )
                        lagged = (kb, p_t)
                        pump(1)
                pk, pt_prev = lagged
                for half in range(2):
                    nc.tensor.matmul(
                        o_ps[0 : D + 1, half * 512 : (half + 1) * 512],
                        lhsT=v_sb[:, rb * 8 + pk, h * (D + 2) : h * (D + 2) + D + 1],
                        rhs=pt_prev[:, half * 512 : (half + 1) * 512],
                        start=(pk == 0),
                        stop=(pk == 7),
                    )
                # fast unnormalized evict releases the o psum slot; the
                # reciprocal chain + in-place normalize run off the critical
                # path.  (l goes to SBUF partition 0 first: rcp_approx_fast
                # miscomputes on a partition-shifted PSUM input.)
                l_sb = rcp_pool.tile([1, T], F32, tag="lsb", name="lsb")
                nc.vector.tensor_copy(l_sb[:], o_ps[D : D + 1, :])
                nc.scalar.copy(oT_sb[hsl, bsl], o_ps[0:D, :])
                rcp_row = rcp_pool.tile([1, T], F32, tag="lsb", name="rrow")
                nc.vector.reciprocal_approx_fast(rcp_row[:], l_sb[:])
                rcp_b = rbc_pool.tile([P, T], F32, tag="rbc", name="rbc")
                nc.gpsimd.partition_broadcast(rcp_b[:], rcp_row[:])
                if DEBUG_TAPS:
                    nc.sync.dma_start(out=dbg["dbg_l"][bh : bh + 1, :], in_=l_sb[:])
                    nc.sync.dma_start(
                        out=dbg["dbg_rcp"][bh : bh + 1, :], in_=rcp_row[:]
                    )
                nc.vector.tensor_mul(
                    oT_sb[hsl, bsl], oT_sb[hsl, bsl].bitcast(F32), rcp_b[hsl, :]
                )
            # output projection for batch b: queued as pump chunks so it
            # fills the next batch's PE gaps (inline for the last batch)
            def outproj_chunks(b=b):
                # et-quads sharing one [P, 4, 512] tile -> 1 MB output DMAs
                for half in range(2):
                    for eq in range(2):

                        def c_out(half=half, eq=eq, b=b):
                            ob = outb_pool.tile([P, 4, 512], F32, tag="ob", name="ob")
                            for ei in range(4):
                                et = eq * 4 + ei
                                psf = pj_psum.tile(
                                    [P, 512], F32, tag="pj", name="psf"
                                )
                                nc.tensor.matmul(
                                    psf[:],
                                    lhsT=wo_sb[:, et, :],
                                    rhs=oT_sb[
                                        :,
                                        (b % 2) * T + half * 512 : (b % 2) * T
                                        + (half + 1) * 512,
                                    ],
                                    start=True,
                                    stop=True,
                                )
                                if et % 2 == 0:
                                    nc.scalar.activation(
                                        ob[:, ei, :],
                                        psf[:],
                                        Identity,
                                        bias=bo_sb[:, et : et + 1],
                                        scale=1.0,
                                    )
                                else:
                                    nc.vector.tensor_scalar_add(
                                        ob[:, ei, :], psf[:], bo_sb[:, et : et + 1]
                                    )
                            nc.sync.dma_start(
                                out=bass.AP(
                                    tensor=outT,
                                    offset=eq * 4 * P * TB + b * T + half * 512,
                                    ap=[[TB, P], [P * TB, 4], [1, 512]],
                                ),
                                in_=ob[:],
                            )

                        yield c_out

            if b < B - 1:
                pending.extend((("out", b), c) for c in outproj_chunks())
            else:
                pump(len(pending))
                for c in outproj_chunks():
                    c()

    nc.compile()
    return nc


_NC_CACHE = None


def _get_nc():
    global _NC_CACHE
    if _NC_CACHE is None:
        _NC_CACHE = _build_bass()
    return _NC_CACHE


def _rope_tables():
    """cos/sin tables in [dim(128, 2 heads stacked), t] layout.

    Rows 0:32 of each 64-row head block carry -sin, rows 32:64 carry +sin
    (the rotate_half signs, indexed by output row: the swapped projection
    supplies qs[d] = q[partner(d)]).  q tables are pre-scaled by the
    attention scale.
    """
    d = np.arange(0, D, 2, dtype=np.float32) / np.float32(D)
    inv_freq = (np.float32(1.0) / np.power(np.float32(10000.0), d)).astype(np.float32)
    t = np.arange(T, dtype=np.float32)
    freqs = t[None, :] * inv_freq[:, None]  # [32, T]
    cos_h = np.cos(np.concatenate([freqs, freqs], axis=0)).astype(np.float32)  # [64,T]
    sin_half = np.sin(freqs).astype(np.float32)
    sin_signed = np.concatenate([-sin_half, sin_half], axis=0)  # [64, T]
    cos = np.vstack([cos_h, cos_h])  # [128, T] (2 heads)
    sin = np.vstack([sin_signed, sin_signed])
    return (np.ascontiguousarray(cos), np.ascontiguousarray(sin))


# partner-row permutation for the swapped projection: within each 64-dim
# head block, row d maps to (d+32) % 64
_SWAP = np.concatenate(
    [np.arange(64).reshape(2, 32)[::-1].ravel() + 64 * hh for hh in range(2)]
)


def _make_in_maps(query, attn_bias, key_padding_mask, Wq, bq, Wk, Wv, bv, Wo, bo):
    query = np.asarray(query, dtype=np.float32)
    attn_bias = np.asarray(attn_bias, dtype=np.float32)
    key_padding_mask = np.asarray(key_padding_mask)
    Wq = np.asarray(Wq, dtype=np.float32)
    Wk = np.asarray(Wk, dtype=np.float32)
    Wv = np.asarray(Wv, dtype=np.float32)
    Wo = np.asarray(Wo, dtype=np.float32)
    bq = np.asarray(bq, dtype=np.float32)
    bv = np.asarray(bv, dtype=np.float32)
    bo = np.asarray(bo, dtype=np.float32)

    # shared across cores
    queryT = np.ascontiguousarray(query.transpose(2, 1, 0).reshape(E, TB))
    masku8 = np.ascontiguousarray(key_padding_mask.astype(np.uint8))
    cos_k, sin_k = _rope_tables()
    bo_zero = np.zeros((P, 8), dtype=np.float32)
    bo_col = np.ascontiguousarray(bo.reshape(8, P).T)  # [p, echunk]

    in_maps = []
    for c in range(NCORES):
        rsl = slice(c * P, (c + 1) * P)
        in_maps.append(
            {
                "queryT": queryT,
                "biasT": np.ascontiguousarray(
                    attn_bias[:, c * HPC : (c + 1) * HPC].transpose(0, 1, 3, 2)
                ).reshape(B * HPC, T, T),
                "wqT": np.ascontiguousarray(Wq[rsl, :].T * np.float32(SCALE)),
                "wkT": np.ascontiguousarray(Wk[rsl, :].T),
                "wqsT": np.ascontiguousarray(Wq[rsl, :][_SWAP, :].T * np.float32(SCALE)),
                "wksT": np.ascontiguousarray(Wk[rsl, :][_SWAP, :].T),
                "wvT": np.ascontiguousarray(Wv[rsl, :].T),
                "woT": np.ascontiguousarray(Wo[:, rsl].T),
                "bq": np.ascontiguousarray(bq[rsl].reshape(P, 1) * np.float32(SCALE)),
                "bqs": np.ascontiguousarray(
                    bq[rsl][_SWAP].reshape(P, 1) * np.float32(SCALE)
                ),
                "bv": np.ascontiguousarray(bv[rsl].reshape(1, P)),
                "bo": bo_col if c == 0 else bo_zero,
                "masku8": masku8,
                "cos_k": cos_k,
                "sin_k": sin_k,
            }
        )
    return in_maps


def _run(inputs, trace=False, **kwargs):
    nc = _get_nc()
    in_maps = _make_in_maps(**inputs)
    res = run_bass_kernel_spmd(
        nc, in_maps, core_ids=list(range(NCORES)), trace=trace, **kwargs
    )
    acc = np.zeros((E, TB), dtype=np.float32)
    for r in res.results:
        acc += r["outT"]
    out = np.ascontiguousarray(acc.reshape(E, B, T).transpose(2, 1, 0))
    return out, res


def kernel(**inputs) -> np.ndarray:
    out, _ = _run(inputs, trace=False)
    return out
